# revision 13
# baseline (speedup 1.0000x reference)
"""MoE layer (8 experts, top-2) on 8 TRN2 NeuronCores, expert-parallel.

V3: host-routed dispatch, device = pure per-expert FFN at PE roofline.

The host computes the (tiny) router matmul [T,8], top-2 gates, and the
per-expert compacted token lists (it already had to do most of this to size
the per-expert capacity). Each core is assigned one expert and receives:
  - its expert's tokens, gathered + transposed to [128, KH, CAP] bf16 on
    the host (default), or gathered on-device via dma_gather + DMA
    transpose (MOE_DEV_GATHER=1),
  - the per-slot combine gates [128, CAP/128] f32,
  - its expert's w1/w2 (bf16, pre-swizzled per-partition-contiguous).
The device runs relu(x@w1+b1)@w2+b2 over the CAP token slots in 512-token
chunks (all matmuls bf16, N=512, K-contiguous, PE stays warm), scales by
the gate, and stores the compacted [CAP, H] bf16 output. The host
scatter-adds the 8 compacted outputs into the full [B,S,H] f32 result.
"""
import os
import sys

for _p in ("/opt/trn_rl_repo", "/root/.axon_site/_ro/trn_rl_repo"):
    if _p not in sys.path:
        sys.path.insert(0, _p)

import numpy as np
import ml_dtypes

import concourse.bass as bass
import concourse.mybir as mybir
import concourse.tile as tile
import concourse.bacc as bacc
from concourse.bass_utils import run_bass_kernel_spmd

BF16 = ml_dtypes.bfloat16
F32 = mybir.dt.float32
BF = mybir.dt.bfloat16

H = 1024          # hidden
F = 2048          # ffn dim
E = 8             # experts
P = 128
TOK_CHUNK = 512   # tokens per FFN chunk
KH = H // P       # k tiles over hidden (8)
KF = F // P       # k tiles over ffn dim (16)
N_CORES = 8

Relu = mybir.ActivationFunctionType.Relu
Alu = mybir.AluOpType


def build_moe_v3(CAP, with_b1, with_b2, dev_gather, T=16384, repeat=1):
    """Device program: per-expert FFN over CAP compacted token slots.

    repeat>1 runs the whole body (weight DMAs included) repeat times —
    used by the timing harness to amortize per-dispatch measurement noise;
    the output is identical.
    """
    NSLOT = CAP // P
    nc = bacc.Bacc("TRN2", target_bir_lowering=False, debug=False,
                   num_devices=N_CORES)

    w1 = nc.declare_dram_parameter("w1", [P, KH * F], BF, isOutput=False)
    w2 = nc.declare_dram_parameter("w2", [P, KF * H], BF, isOutput=False)
    b1v = nc.declare_dram_parameter("b1v", [P, KF], F32, isOutput=False)
    b2bc = nc.declare_dram_parameter("b2bc", [P, H], F32, isOutput=False)
    gates = nc.declare_dram_parameter("gates", [P, NSLOT], F32, isOutput=False)
    if dev_gather:
        xrows = nc.declare_dram_parameter("xrows", [T, H], BF, isOutput=False)
        gidx = nc.declare_dram_parameter("gidx", [P, CAP // 16],
                                         mybir.dt.int16, isOutput=False)
    else:
        xg_in = nc.declare_dram_parameter("xg", [P, KH * CAP], BF,
                                          isOutput=False)
        xg_v = xg_in.rearrange("p (ko t) -> p ko t", ko=KH)
    out = nc.declare_dram_parameter("yout", [CAP, H], BF, isOutput=True)

    w1_v = w1.rearrange("p (ko f) -> p ko f", ko=KH)
    w2_v = w2.rearrange("p (ko h) -> p ko h", ko=KF)
    out_v = out.rearrange("(n p) h -> p n h", p=P)

    # chunk sizes: full TOK_CHUNKs plus a 128-granular remainder
    sizes = []
    left = CAP
    while left > 0:
        s = min(TOK_CHUNK, left)
        sizes.append(s)
        left -= s

    with tile.TileContext(nc) as tc:
        with (
            tc.tile_pool(name="weights", bufs=1) as wpool,
            tc.tile_pool(name="xg", bufs=4) as xgpool,
            tc.tile_pool(name="xr", bufs=3) as xrpool,
            tc.tile_pool(name="ht", bufs=3) as htpool,
            tc.tile_pool(name="osb", bufs=3) as opool,
            tc.tile_pool(name="psum_h", bufs=4, space="PSUM") as phpool,
            tc.tile_pool(name="psum_y", bufs=4, space="PSUM") as pypool,
        ):
          for _rep in range(repeat):
            # chunk-0 activations are queued before the (much larger)
            # weight DMAs so the first w1 group can start ~20us earlier
            xg0 = xgpool.tile([P, KH, sizes[0]], BF, tag="xg")
            if not dev_gather:
                nc.sync.dma_start(xg0[:], xg_v[:, :, 0:sizes[0]])
            w1_sb = wpool.tile([P, KH, F], BF)
            # split the w1 load so the first ft-groups' slices land early
            # and chunk-0 compute starts ~20us sooner
            for q in range(4):
                nc.sync.dma_start(
                    w1_sb[:, :, q * (F // 4):(q + 1) * (F // 4)],
                    w1_v[:, :, q * (F // 4):(q + 1) * (F // 4)])
            w2_sb = wpool.tile([P, KF, H], BF)
            nc.sync.dma_start(w2_sb[:], w2_v[:])
            b1_sb = wpool.tile([P, KF], F32)
            nc.sync.dma_start(b1_sb[:], b1v[:])
            b2_bc = None
            if with_b2:
                b2_bc = wpool.tile([P, H], F32)
                nc.sync.dma_start(b2_bc[:], b2bc[:])
            gates_sb = wpool.tile([P, NSLOT], F32)
            nc.sync.dma_start(gates_sb[:], gates[:])
            if dev_gather:
                gidx_sb = wpool.tile([P, CAP // 16], mybir.dt.int16)
                nc.sync.dma_start(gidx_sb[:], gidx[:])

            base = 0
            for c, SZ in enumerate(sizes):
                cn = SZ // P
                xg = xg0 if c == 0 else xgpool.tile([P, KH, SZ], BF, tag="xg")
                if dev_gather:
                    # plain (non-transposed) gather: 2KB/row descriptors,
                    # token t -> partition t%128, row t//128
                    xr = xrpool.tile([P, cn, H], BF, tag="xr")
                    nc.gpsimd.dma_gather(
                        out_ap=xr[:],
                        in_ap=xrows[:, :],
                        idxs_ap=gidx_sb[:, base // 16:(base + SZ) // 16],
                        num_idxs=SZ,
                        num_idxs_reg=SZ,
                        elem_size=H,
                    )
                    # xbar transpose each 128-token row into hidden-major
                    for r in range(cn):
                        nc.sync.dma_start(
                            xg[:, :, r * P:(r + 1) * P],
                            xr[:, r, :],
                            transpose=True,
                        )
                elif c != 0:
                    nc.sync.dma_start(xg[:], xg_v[:, :, base:base + SZ])

                hT = htpool.tile([P, KF, SZ], BF, tag="hT")
                for ft in range(KF):
                    ph = phpool.tile([P, SZ], F32, tag="ph")
                    for k in range(KH):
                        nc.tensor.matmul(
                            ph[:],
                            w1_sb[:, k, ft * P:(ft + 1) * P],
                            xg[:, k, :],
                            start=(k == 0), stop=(k == KH - 1),
                        )
                    if with_b1 or ft % 2:
                        nc.scalar.activation(hT[:, ft, :], ph[:], Relu,
                                             bias=b1_sb[:, ft:ft + 1])
                    else:
                        # b1 == 0: alternate relu between ACT and DVE so
                        # neither engine's per-op overhead paces the w1 phase
                        nc.vector.tensor_scalar_max(hT[:, ft, :], ph[:], 0.0)

                osb = opool.tile([P, cn, H], BF, tag="osb")
                for tt in range(cn):
                    st = base // P + tt
                    gate = gates_sb[:, st:st + 1]
                    py0 = pypool.tile([P, 512], F32, tag="py")
                    py1 = pypool.tile([P, 512], F32, tag="py")
                    pys = [py0, py1]
                    # nh outer: mono-bank 16-MM accumulation chains measure
                    # ~220 ns/MM vs ~236 for bank-alternating pairs
                    for nh in range(2):
                        for k in range(KF):
                            nc.tensor.matmul(
                                pys[nh][:],
                                hT[:, k, tt * P:(tt + 1) * P],
                                w2_sb[:, k, nh * 512:(nh + 1) * 512],
                                start=(k == 0), stop=(k == KF - 1),
                            )
                    for nh in range(2):
                        dst = osb[:, tt, nh * 512:(nh + 1) * 512]
                        if with_b2:
                            nc.vector.tensor_tensor(
                                dst, pys[nh][:],
                                b2_bc[:, nh * 512:(nh + 1) * 512], Alu.add)
                            nc.vector.tensor_scalar_mul(dst, dst, gate)
                        else:
                            nc.vector.tensor_scalar_mul(dst, pys[nh][:], gate)
                nc.sync.dma_start(
                    out_v[:, base // P:(base + SZ) // P, :], osb[:])
                base += SZ

    nc.compile()
    return nc


_NC_CACHE = {}


def get_nc(CAP, with_b1, with_b2, dev_gather, repeat=1):
    key = (CAP, with_b1, with_b2, dev_gather, repeat)
    if key not in _NC_CACHE:
        _NC_CACHE[key] = build_moe_v3(CAP, with_b1, with_b2, dev_gather,
                                      repeat=repeat)
    return _NC_CACHE[key]


def host_route(x2, router_w, router_b):
    """Top-2 routing on host (fp32 logits like the reference, fp64 gates).

    Returns (toks, gats, CAP): per-expert padded token-id arrays [E, CAP]
    int32 and gate arrays [E, CAP] f32; padding slots have gate 0.0.
    """
    T = x2.shape[0]
    lg = x2.astype(np.float32) @ router_w.astype(np.float32)
    lg = lg + router_b.astype(np.float32)
    i1 = np.argmax(lg, axis=1)
    l1 = lg[np.arange(T), i1]
    lg2 = lg.copy()
    lg2[np.arange(T), i1] = -np.inf
    i2 = np.argmax(lg2, axis=1)
    l2 = lg2[np.arange(T), i2]
    e2 = np.exp(l2.astype(np.float64) - l1.astype(np.float64))
    g1 = 1.0 / (1.0 + e2)
    g2 = e2 / (1.0 + e2)

    counts = np.bincount(i1, minlength=E) + np.bincount(i2, minlength=E)
    CAP = max(P, int(np.ceil(counts.max() / P)) * P)
    toks = np.zeros((E, CAP), np.int32)
    gats = np.zeros((E, CAP), np.float32)
    for e in range(E):
        t1 = np.nonzero(i1 == e)[0]
        t2 = np.nonzero(i2 == e)[0]
        te = np.concatenate([t1, t2])
        ge = np.concatenate([g1[t1], g2[t2]]).astype(np.float32)
        toks[e, :len(te)] = te
        gats[e, :len(te)] = ge
    return toks, gats, CAP


def _sw(a, ko):
    # [ko*128, n] -> [128, ko*n] per-partition-contiguous
    n = a.shape[1]
    return np.ascontiguousarray(
        a.reshape(ko, P, n).transpose(1, 0, 2).reshape(P, ko * n))


def prep_inputs_v3(x, router_w, router_b, w1, b1, w2, b2,
                   dev_gather=None):
    """Returns (in_maps, toks, gats, CAP, dev_gather)."""
    if dev_gather is None:
        dev_gather = bool(int(os.environ.get("MOE_DEV_GATHER", "0")))
    T = x.shape[0] * x.shape[1] if x.ndim == 3 else x.shape[0]
    x2 = np.ascontiguousarray(np.asarray(x).reshape(T, H))
    toks, gats, CAP = host_route(x2, np.asarray(router_w),
                                 np.asarray(router_b))
    NSLOT = CAP // P
    x2bf = x2.astype(BF16)
    in_maps = []
    for e in range(E):
        m = {
            "w1": _sw(np.ascontiguousarray(w1[e]).astype(BF16), KH),
            "w2": _sw(np.ascontiguousarray(w2[e]).astype(BF16), KF),
            "b1v": np.ascontiguousarray(
                b1[e].reshape(KF, P).T).astype(np.float32),
            "b2bc": np.tile(b2[e].reshape(1, H).astype(np.float32), (P, 1)),
            # slot s = st*128 + p  ->  gates[p, st]
            "gates": np.ascontiguousarray(
                gats[e].reshape(NSLOT, P).T).astype(np.float32),
        }
        if dev_gather:
            m["xrows"] = x2bf
            # wrapped idx layout: slot s at [s%16, s//16], replicated over
            # the 8 16-partition groups
            gi = toks[e].astype(np.int16).reshape(CAP // 16, 16).T
            m["gidx"] = np.ascontiguousarray(np.tile(gi, (P // 16, 1)))
        else:
            # gather+transpose on host: [P, KH*CAP] hidden-major
            g = x2bf[toks[e]]                       # [CAP, H]
            gT = np.ascontiguousarray(g.T)          # [H, CAP]
            m["xg"] = _sw(gT, KH)
        in_maps.append(m)
    return in_maps, toks, gats, CAP, dev_gather


def kernel(x, router_w, router_b, w1, b1, w2, b2):
    x = np.asarray(x); router_w = np.asarray(router_w)
    router_b = np.asarray(router_b)
    w1 = np.asarray(w1); b1 = np.asarray(b1)
    w2 = np.asarray(w2); b2 = np.asarray(b2)
    B, S, _ = x.shape
    T = B * S
    with_b1 = bool(np.any(b1))
    with_b2 = bool(np.any(b2))
    in_maps, toks, gats, CAP, dev_gather = prep_inputs_v3(
        x, router_w, router_b, w1, b1, w2, b2)
    nc = get_nc(CAP, with_b1, with_b2, dev_gather)
    res = run_bass_kernel_spmd(nc, in_maps, list(range(N_CORES)))
    # numpy fancy += drops duplicate-index contributions, so padding slots
    # (token id 0, gate 0) must not collide with a genuine token-0 slot:
    # route them to a dump row T. Within one expert genuine tokens are
    # unique (top-2 expert ids are distinct), so no other duplicates exist.
    acc = np.zeros((T + 1, H), np.float32)
    for e in range(E):
        y = np.asarray(res.results[e]["yout"]).astype(np.float32)  # [CAP, H]
        ctoks = np.where(gats[e] > 0, toks[e].astype(np.int64), T)
        acc[ctoks] += y
    return acc[:T].reshape(B, S, H)
